# revision 8
# baseline (speedup 1.0000x reference)
"""JambaBlock Trainium2 kernel: 8-core SPMD bass/Tile implementation.

Sharding:
  - mamba in_proj_x/conv/scan: channel-sharded (256 of 2048 d_inner per core)
  - x_proj partials -> AllReduce [96, 4096] (1.6 MB)
  - scan y -> AllToAll [8, 256, 512] to token-shard (4.2 MB)
  - z_proj / out_proj / LN2 / router / MoE: token-sharded (512 of 4096 per core)
  - MoE: dense all-8-experts in bf16, fp32 router, weighted accumulate
  - output: per-core [512, 1024] slice, host concat
"""
import sys

sys.path.insert(0, "/opt/trn_rl_repo")

import numpy as np
import ml_dtypes

import concourse.bass as bass
import concourse.bacc as bacc
import concourse.mybir as mybir
import concourse.tile as tile
from concourse.bass_utils import run_bass_kernel_spmd
from concourse.masks import make_identity

FP32 = mybir.dt.float32
BF16 = mybir.dt.bfloat16
ALU = mybir.AluOpType
ACTF = mybir.ActivationFunctionType

NCORE = 8
D = 1024
DI = 2048
CH = DI // NCORE  # 256 channels per core
DS = 16
DTR = 64
Bb = 2
T = 2048
NTOK = Bb * T  # 4096
TPC = NTOK // NCORE  # 512 tokens per core
HID = 2048
NEXP = 8
EPS = 1e-5
P = 128
MT = CH // P  # 2 channel tiles per core
TCH = 1024  # scan time-chunk
NCH = T // TCH  # 2 chunks per batch seq


def _bcast_rows(nc, dst, src_row):
    """Replicate src_row [1, W] across dst [128, W] via doubling DMAs."""
    nc.sync.dma_start(out=dst[0:1, :], in_=src_row)
    p = 1
    while p < P:
        nc.sync.dma_start(out=dst[p : 2 * p, :], in_=dst[0:p, :])
        p *= 2


def _layernorm_tile(nc, pool, xt, s_bc, b_bc, out_t, zc, ec):
    """LN over free dim of xt [128, D] -> out_t (fp32). s_bc/b_bc [128, D]."""
    s1 = pool.tile([P, 1], FP32, name="ln_s1", tag="ln_s1", bufs=2)
    nm = pool.tile([P, 1], FP32, name="ln_nm", tag="ln_nm", bufs=2)
    nc.vector.tensor_reduce(s1[:], xt[:], axis=mybir.AxisListType.X, op=ALU.add)
    nc.vector.tensor_scalar(
        out=nm[:], in0=s1[:], scalar1=-1.0 / D, scalar2=None, op0=ALU.mult
    )
    xc = pool.tile([P, D], FP32, name="ln_xc", tag="ln_xc", bufs=2)
    nc.scalar.activation(xc[:], xt[:], ACTF.Identity, bias=nm[:])
    sq = pool.tile([P, D], FP32, name="ln_sq", tag="ln_sq", bufs=2)
    ssq = pool.tile([P, 1], FP32, name="ln_ssq", tag="ln_ssq", bufs=2)
    nc.scalar.activation(sq[:], xc[:], ACTF.Square, bias=zc[:], accum_out=ssq[:])
    sd = pool.tile([P, 1], FP32, name="ln_sd", tag="ln_sd", bufs=2)
    nc.scalar.activation(sd[:], ssq[:], ACTF.Sqrt, scale=1.0 / D, bias=ec[:])
    r = pool.tile([P, 1], FP32, name="ln_r", tag="ln_r", bufs=2)
    nc.vector.reciprocal(r[:], sd[:])
    t2 = pool.tile([P, D], FP32, name="ln_t2", tag="ln_t2", bufs=2)
    nc.vector.scalar_tensor_tensor(
        out=t2[:], in0=xc[:], scalar=r[:], in1=s_bc[:], op0=ALU.mult, op1=ALU.mult
    )
    nc.vector.tensor_tensor(out_t[:], t2[:], b_bc[:], op=ALU.add)


def build():
    nc = bacc.Bacc(None, target_bir_lowering=False, debug=False)

    # ---- I/O -------------------------------------------------------------
    x_full = nc.declare_dram_parameter("x_full", [NTOK, D], FP32, isOutput=False)
    x_my = nc.declare_dram_parameter("x_my", [TPC, D], FP32, isOutput=False)
    w_in_x = nc.declare_dram_parameter("w_in_x", [P, 8, CH], FP32, isOutput=False)
    w_in_z = nc.declare_dram_parameter("w_in_z", [P, 8, DI], FP32, isOutput=False)
    w_xp = nc.declare_dram_parameter("w_xp", [P, MT, 96], FP32, isOutput=False)
    w_dt = nc.declare_dram_parameter("w_dt", [DTR, CH], FP32, isOutput=False)
    w_out = nc.declare_dram_parameter("w_out", [P, 16, D], FP32, isOutput=False)
    w_rt = nc.declare_dram_parameter("w_rt", [P, 8, NEXP], FP32, isOutput=False)
    wg = nc.declare_dram_parameter("wg", [NEXP, P, 8, HID], BF16, isOutput=False)
    wu = nc.declare_dram_parameter("wu", [NEXP, P, 8, HID], BF16, isOutput=False)
    wd = nc.declare_dram_parameter("wd", [NEXP, P, 16, D], BF16, isOutput=False)
    conv_w = nc.declare_dram_parameter("conv_w", [P, MT, 4], FP32, isOutput=False)
    conv_b = nc.declare_dram_parameter("conv_b", [P, MT], FP32, isOutput=False)
    dt_bias = nc.declare_dram_parameter("dt_bias", [P, MT], FP32, isOutput=False)
    dp_in = nc.declare_dram_parameter("dp", [P, MT], FP32, isOutput=False)
    a_log = nc.declare_dram_parameter("a_log", [P, MT, DS], FP32, isOutput=False)
    ssm0 = nc.declare_dram_parameter("ssm0", [Bb, MT, P, DS], FP32, isOutput=False)
    conv0 = nc.declare_dram_parameter("conv0", [Bb, MT, P, 3], FP32, isOutput=False)
    ln1_s = nc.declare_dram_parameter("ln1_s", [1, D], FP32, isOutput=False)
    ln1_b = nc.declare_dram_parameter("ln1_b", [1, D], FP32, isOutput=False)
    ln2_s = nc.declare_dram_parameter("ln2_s", [1, D], FP32, isOutput=False)
    ln2_b = nc.declare_dram_parameter("ln2_b", [1, D], FP32, isOutput=False)
    out_p = nc.declare_dram_parameter("out", [TPC, D], FP32, isOutput=True)

    rg = [list(range(NCORE))]

    with tile.TileContext(nc) as tc:
        with tc.tile_pool(name="dram", bufs=1, space="DRAM") as dpool:
            xcs_d = dpool.tile([MT, P, NTOK], FP32, name="xcs_d")
            xz_d = dpool.tile([MT, P, NTOK], FP32, name="xz_d")
            z_d = dpool.tile([16, P, TPC], FP32, name="z_d")
            dt_d = dpool.tile([MT, P, NTOK], FP32, name="dt_d")
            ar_in = dpool.tile([96, NTOK], FP32, name="ar_in")
            ar_out = dpool.tile([96, NTOK], FP32, name="ar_out", addr_space="Shared")
            a2a_in = dpool.tile([NCORE, CH, TPC], FP32, name="a2a_in")
            a2a_out = dpool.tile([NCORE, CH, TPC], FP32, name="a2a_out")

            with tc.tile_pool(name="per", bufs=1) as per:
                # ---- persistent small tensors -----------------------------
                ident = per.tile([P, P], FP32, name="ident")
                make_identity(nc, ident)
                zero_col = per.tile([P, 1], FP32, name="zero_col")
                nc.vector.memset(zero_col[:], 0.0)
                eps_col = per.tile([P, 1], FP32, name="eps_col")
                nc.vector.memset(eps_col[:], EPS)
                one_col = per.tile([P, 1], FP32, name="one_col")
                nc.vector.memset(one_col[:], 1.0)
                s1b = per.tile([P, D], FP32, name="s1b")
                b1b = per.tile([P, D], FP32, name="b1b")
                s2b = per.tile([P, D], FP32, name="s2b")
                b2b = per.tile([P, D], FP32, name="b2b")
                _bcast_rows(nc, s1b, ln1_s[:])
                _bcast_rows(nc, b1b, ln1_b[:])
                _bcast_rows(nc, s2b, ln2_s[:])
                _bcast_rows(nc, b2b, ln2_b[:])
                cw_sb = per.tile([P, MT, 4], FP32, name="cw_sb")
                nc.sync.dma_start(out=cw_sb[:], in_=conv_w[:])
                cb_sb = per.tile([P, MT], FP32, name="cb_sb")
                nc.sync.dma_start(out=cb_sb[:], in_=conv_b[:])
                dtb_sb = per.tile([P, MT], FP32, name="dtb_sb")
                nc.sync.dma_start(out=dtb_sb[:], in_=dt_bias[:])
                dp_sb = per.tile([P, MT], FP32, name="dp_sb")
                nc.sync.dma_start(out=dp_sb[:], in_=dp_in[:])
                al_sb = per.tile([P, MT, DS], FP32, name="al_sb")
                nc.sync.dma_start(out=al_sb[:], in_=a_log[:])
                a_sb = per.tile([P, MT, DS], FP32, name="a_sb")
                # A = -exp(A_log)
                nc.scalar.activation(
                    a_sb[:, :, :].rearrange("p a b -> p (a b)"),
                    al_sb[:, :, :].rearrange("p a b -> p (a b)"),
                    ACTF.Exp,
                    bias=zero_col[:],
                )
                nc.vector.tensor_scalar(
                    out=a_sb[:, :, :].rearrange("p a b -> p (a b)"),
                    in0=a_sb[:, :, :].rearrange("p a b -> p (a b)"),
                    scalar1=-1.0,
                    scalar2=None,
                    op0=ALU.mult,
                )
                w_xp_sb = per.tile([P, MT, 96], FP32, name="w_xp_sb")
                nc.sync.dma_start(out=w_xp_sb[:], in_=w_xp[:])
                w_dt_sb = per.tile([DTR, CH], FP32, name="w_dt_sb")
                nc.sync.dma_start(out=w_dt_sb[:], in_=w_dt[:])
                xmy_sb = per.tile([P, 4, D], FP32, name="xmy_sb")
                nc.sync.dma_start(
                    out=xmy_sb[:],
                    in_=x_my.rearrange("(a p) d -> p a d", p=P),
                )
                # proj (x_proj output after AllReduce) [96, NTOK]
                proj_sb = per.tile([96, NTOK], FP32, name="proj_sb")
                # x1 (also the final accumulator), token-major
                x1_sb = per.tile([P, 4, D], FP32, name="x1_sb")

                # ======== PHASE 1a: LN1 + transposes + in_proj_x + z_proj ===
                with tc.tile_pool(name="p1", bufs=1) as p1, tc.tile_pool(
                    name="psT", bufs=2, space="PSUM"
                ) as psT, tc.tile_pool(name="psA", bufs=2, space="PSUM") as psA:
                    w_in_x_sb = p1.tile([P, 8, CH], FP32, name="w_in_x_sb")
                    nc.sync.dma_start(out=w_in_x_sb[:], in_=w_in_x[:])

                    for ns in range(8):  # 512-token slices
                        xn_fm = p1.tile(
                            [P, 8, 512], FP32, name="xn_fm", tag="xn_fm", bufs=2
                        )
                        for tt in range(4):
                            row0 = (ns * 4 + tt) * P
                            xt = p1.tile([P, D], FP32, name="xt", tag="xt", bufs=3)
                            nc.sync.dma_start(
                                out=xt[:], in_=x_full[row0 : row0 + P, :]
                            )
                            xn_t = p1.tile(
                                [P, D], FP32, name="xn_t", tag="xn_t", bufs=2
                            )
                            _layernorm_tile(nc, p1, xt, s1b, b1b, xn_t, zero_col, eps_col)
                            for k in range(8):
                                tp = psT.tile([P, P], FP32, name="tp", tag="tp")
                                nc.tensor.transpose(
                                    tp[:], xn_t[:, k * P : (k + 1) * P], ident[:]
                                )
                                eng = nc.scalar if (k % 2 == 0) else nc.vector
                                if eng is nc.scalar:
                                    nc.scalar.copy(
                                        xn_fm[:, k, tt * P : (tt + 1) * P], tp[:]
                                    )
                                else:
                                    nc.vector.tensor_copy(
                                        xn_fm[:, k, tt * P : (tt + 1) * P], tp[:]
                                    )
                        # in_proj_x for this token slice
                        for m in range(MT):
                            ps = psA.tile([P, 512], FP32, name="ps_inx", tag="ps_inx")
                            for k in range(8):
                                nc.tensor.matmul(
                                    ps[:],
                                    w_in_x_sb[:, k, m * P : (m + 1) * P],
                                    xn_fm[:, k, :],
                                    start=(k == 0),
                                    stop=(k == 7),
                                )
                            xz_t = p1.tile(
                                [P, 512], FP32, name="xz_t", tag="xz_t", bufs=2
                            )
                            nc.scalar.copy(xz_t[:], ps[:])
                            nc.sync.dma_start(
                                out=xz_d[m, :, ns * 512 : (ns + 1) * 512], in_=xz_t[:]
                            )

                    # z_proj: token slice of this core only
                    xnz_fm = p1.tile([P, 8, 512], FP32, name="xnz_fm")
                    for tt in range(4):
                        xn_t = p1.tile([P, D], FP32, name="xn_t", tag="xn_t", bufs=2)
                        _layernorm_tile(nc, p1, xmy_sb[:, tt, :], s1b, b1b, xn_t, zero_col, eps_col)
                        for k in range(8):
                            tp = psT.tile([P, P], FP32, name="tp", tag="tp")
                            nc.tensor.transpose(
                                tp[:], xn_t[:, k * P : (k + 1) * P], ident[:]
                            )
                            if k % 2 == 0:
                                nc.scalar.copy(
                                    xnz_fm[:, k, tt * P : (tt + 1) * P], tp[:]
                                )
                            else:
                                nc.vector.tensor_copy(
                                    xnz_fm[:, k, tt * P : (tt + 1) * P], tp[:]
                                )
                    for m2 in range(16):
                        wz_m = p1.tile(
                            [P, 8, P], FP32, name="wz_m", tag="wz_m", bufs=3
                        )
                        nc.sync.dma_start(
                            out=wz_m[:], in_=w_in_z[:, :, m2 * P : (m2 + 1) * P]
                        )
                        ps = psA.tile([P, 512], FP32, name="ps_z", tag="ps_z")
                        for k in range(8):
                            nc.tensor.matmul(
                                ps[:],
                                wz_m[:, k, :],
                                xnz_fm[:, k, :],
                                start=(k == 0),
                                stop=(k == 7),
                            )
                        zt = p1.tile([P, TPC], FP32, name="zt", tag="zt", bufs=2)
                        nc.scalar.activation(zt[:], ps[:], ACTF.Silu, bias=zero_col[:])
                        nc.sync.dma_start(out=z_d[m2, :, :], in_=zt[:])

                # ---- conv (depthwise causal, taps=4) + SiLU -> xcs_d ---
                with tc.tile_pool(name="p1b", bufs=1) as p1, tc.tile_pool(
                    name="psB", bufs=2, space="PSUM"
                ) as psA:
                    for b in range(Bb):
                        for m in range(MT):
                            xpad = p1.tile(
                                [P, T + 3], FP32, name="xpad", tag="xpad", bufs=2
                            )
                            nc.sync.dma_start(
                                out=xpad[:, 0:3], in_=conv0[b, m, :, :]
                            )
                            nc.sync.dma_start(
                                out=xpad[:, 3:],
                                in_=xz_d[m, :, b * T : (b + 1) * T],
                            )
                            cacc = p1.tile(
                                [P, T], FP32, name="cacc", tag="cacc", bufs=2
                            )
                            nc.vector.tensor_scalar(
                                out=cacc[:],
                                in0=xpad[:, 0:T],
                                scalar1=cw_sb[:, m, 0:1],
                                scalar2=None,
                                op0=ALU.mult,
                            )
                            for k in range(1, 4):
                                nc.vector.scalar_tensor_tensor(
                                    out=cacc[:],
                                    in0=xpad[:, k : k + T],
                                    scalar=cw_sb[:, m, k : k + 1],
                                    in1=cacc[:],
                                    op0=ALU.mult,
                                    op1=ALU.add,
                                )
                            xcs_t = p1.tile(
                                [P, T], FP32, name="xcs_t", tag="xcs_t", bufs=2
                            )
                            nc.scalar.activation(
                                xcs_t[:], cacc[:], ACTF.Silu, bias=cb_sb[:, m : m + 1]
                            )
                            nc.sync.dma_start(
                                out=xcs_d[m, :, b * T : (b + 1) * T], in_=xcs_t[:]
                            )

                    # ---- x_proj partials + AllReduce -----------------------
                    for n in range(8):
                        ps = psA.tile([96, 512], FP32, name="ps_xp", tag="ps_xp")
                        for k in range(MT):
                            xc_kn = p1.tile(
                                [P, 512], FP32, name="xc_kn", tag="xc_kn", bufs=3
                            )
                            nc.sync.dma_start(
                                out=xc_kn[:],
                                in_=xcs_d[k, :, n * 512 : (n + 1) * 512],
                            )
                            nc.tensor.matmul(
                                ps[:],
                                w_xp_sb[:, k, :],
                                xc_kn[:],
                                start=(k == 0),
                                stop=(k == MT - 1),
                            )
                        pp_t = p1.tile([96, 512], FP32, name="pp_t", tag="pp_t", bufs=2)
                        nc.vector.tensor_copy(pp_t[:], ps[:])
                        nc.sync.dma_start(
                            out=ar_in[:, n * 512 : (n + 1) * 512], in_=pp_t[:]
                        )
                    nc.gpsimd.collective_compute(
                        "AllReduce",
                        ALU.add,
                        replica_groups=rg,
                        ins=[ar_in.opt()],
                        outs=[ar_out.opt()],
                    )
                    nc.sync.dma_start(out=proj_sb[:], in_=ar_out[:])

                    # ---- dt = softplus(dtraw @ w_dt + b) -------------------
                    for m in range(MT):
                        for n in range(8):
                            ps = psA.tile([P, 512], FP32, name="ps_dt", tag="ps_dt")
                            nc.tensor.matmul(
                                ps[:],
                                w_dt_sb[:, m * P : (m + 1) * P],
                                proj_sb[0:DTR, n * 512 : (n + 1) * 512],
                                start=True,
                                stop=True,
                            )
                            et = p1.tile([P, 512], FP32, name="et", tag="et", bufs=2)
                            nc.scalar.activation(
                                et[:], ps[:], ACTF.Exp, bias=dtb_sb[:, m : m + 1]
                            )
                            dt_t = p1.tile(
                                [P, 512], FP32, name="dt_t", tag="dt_t", bufs=2
                            )
                            nc.scalar.activation(dt_t[:], et[:], ACTF.Ln, bias=one_col[:])
                            nc.sync.dma_start(
                                out=dt_d[m, :, n * 512 : (n + 1) * 512], in_=dt_t[:]
                            )

                # ======== PHASE 1b: the selective scan ======================
                with tc.tile_pool(name="sc", bufs=1) as sc:
                    for b in range(Bb):
                        dt_u = []
                        u_u = []
                        y_acc = []
                        hstate = []
                        for m in range(MT):
                            dtt = sc.tile(
                                [P, T], FP32, name=f"dt_u{m}", tag=f"dt_u{m}", bufs=1
                            )
                            nc.sync.dma_start(
                                out=dtt[:], in_=dt_d[m, :, b * T : (b + 1) * T]
                            )
                            dt_u.append(dtt)
                            xcst = sc.tile(
                                [P, T], FP32, name=f"xc_u{m}", tag=f"xc_u{m}", bufs=1
                            )
                            nc.sync.dma_start(
                                out=xcst[:], in_=xcs_d[m, :, b * T : (b + 1) * T]
                            )
                            ut = sc.tile(
                                [P, T], FP32, name=f"u_u{m}", tag=f"u_u{m}", bufs=1
                            )
                            nc.vector.tensor_tensor(ut[:], dtt[:], xcst[:], op=ALU.mult)
                            u_u.append(ut)
                            hs = sc.tile(
                                [P, DS], FP32, name=f"hs{m}", tag=f"hs{m}", bufs=2
                            )
                            nc.sync.dma_start(out=hs[:], in_=ssm0[b, m, :, :])
                            hstate.append(hs)
                            ya = sc.tile(
                                [P, T], FP32, name=f"ya{m}", tag=f"ya{m}", bufs=1
                            )
                            y_acc.append(ya)

                        for c in range(NCH):
                            t0 = c * TCH
                            for ds in range(DS):
                                bbt = sc.tile(
                                    [P, TCH], FP32, name="bbt", tag="bbt", bufs=2
                                )
                                _bcast_rows(
                                    nc,
                                    bbt,
                                    proj_sb[
                                        64 + ds : 65 + ds,
                                        b * T + t0 : b * T + t0 + TCH,
                                    ],
                                )
                                cbt = sc.tile(
                                    [P, TCH], FP32, name="cbt", tag="cbt", bufs=2
                                )
                                _bcast_rows(
                                    nc,
                                    cbt,
                                    proj_sb[
                                        80 + ds : 81 + ds,
                                        b * T + t0 : b * T + t0 + TCH,
                                    ],
                                )
                                for m in range(MT):
                                    dA = sc.tile(
                                        [P, TCH], FP32, name="dA", tag="dA", bufs=2
                                    )
                                    nc.scalar.activation(
                                        dA[:],
                                        dt_u[m][:, t0 : t0 + TCH],
                                        ACTF.Exp,
                                        bias=zero_col[:],
                                        scale=a_sb[:, m, ds : ds + 1],
                                    )
                                    dBx = sc.tile(
                                        [P, TCH], FP32, name="dBx", tag="dBx", bufs=2
                                    )
                                    nc.vector.tensor_tensor(
                                        dBx[:],
                                        u_u[m][:, t0 : t0 + TCH],
                                        bbt[:],
                                        op=ALU.mult,
                                    )
                                    h = sc.tile(
                                        [P, TCH], FP32, name="h", tag="h", bufs=2
                                    )
                                    nc.vector.tensor_tensor_scan(
                                        h[:],
                                        dA[:],
                                        dBx[:],
                                        hstate[m][:, ds : ds + 1],
                                        op0=ALU.mult,
                                        op1=ALU.add,
                                    )
                                    # save final state for chunk chaining
                                    nc.vector.tensor_copy(
                                        hstate[m][:, ds : ds + 1], h[:, TCH - 1 : TCH]
                                    )
                                    if ds == 0:
                                        nc.vector.tensor_tensor(
                                            y_acc[m][:, t0 : t0 + TCH],
                                            h[:],
                                            cbt[:],
                                            op=ALU.mult,
                                        )
                                    else:
                                        hC = sc.tile(
                                            [P, TCH], FP32, name="hC", tag="hC", bufs=2
                                        )
                                        nc.gpsimd.tensor_tensor(
                                            hC[:], h[:], cbt[:], op=ALU.mult
                                        )
                                        nc.vector.tensor_tensor(
                                            y_acc[m][:, t0 : t0 + TCH],
                                            y_acc[m][:, t0 : t0 + TCH],
                                            hC[:],
                                            op=ALU.add,
                                        )
                        # add Dp * xcs, then ship chunks to a2a_in
                        for m in range(MT):
                            xcst = sc.tile(
                                [P, T], FP32, name=f"xc2_{m}", tag=f"xc_u{m}", bufs=1
                            )
                            nc.sync.dma_start(
                                out=xcst[:], in_=xcs_d[m, :, b * T : (b + 1) * T]
                            )
                            nc.vector.scalar_tensor_tensor(
                                out=y_acc[m][:],
                                in0=xcst[:],
                                scalar=dp_sb[:, m : m + 1],
                                in1=y_acc[m][:],
                                op0=ALU.mult,
                                op1=ALU.add,
                            )
                            for tc4 in range(4):
                                j = b * 4 + tc4
                                nc.sync.dma_start(
                                    out=a2a_in[j, m * P : (m + 1) * P, :],
                                    in_=y_acc[m][:, tc4 * TPC : (tc4 + 1) * TPC],
                                )

                nc.gpsimd.collective_compute(
                    "AllToAll",
                    ALU.bypass,
                    replica_groups=rg,
                    ins=[a2a_in.opt()],
                    outs=[a2a_out.opt()],
                )

                # ======== PHASE 1c: ymul + out_proj + residual ==============
                with tc.tile_pool(name="op", bufs=1) as op, tc.tile_pool(
                    name="psOP", bufs=1, space="PSUM"
                ) as psOP:
                    ym_sb = op.tile([P, 16, TPC], FP32, name="ym_sb")
                    for r in range(NCORE):
                        nc.sync.dma_start(
                            out=ym_sb[:, 2 * r, :], in_=a2a_out[r, 0:P, :]
                        )
                        nc.sync.dma_start(
                            out=ym_sb[:, 2 * r + 1, :], in_=a2a_out[r, P:CH, :]
                        )
                    # ymul = y * silu(z)
                    for kz in range(16):
                        zt2 = op.tile([P, TPC], FP32, name="zt2", tag="zt2", bufs=3)
                        nc.sync.dma_start(out=zt2[:], in_=z_d[kz, :, :])
                        nc.vector.tensor_tensor(
                            ym_sb[:, kz, :], ym_sb[:, kz, :], zt2[:], op=ALU.mult
                        )
                    for n2 in range(2):
                        ps_l = [
                            psOP.tile([P, 512], FP32, name=f"psop{mt}", tag=f"psop{mt}")
                            for mt in range(4)
                        ]
                        for k in range(16):
                            wo_kt = op.tile(
                                [P, 512], FP32, name="wo_kt", tag="wo_kt", bufs=3
                            )
                            nc.sync.dma_start(
                                out=wo_kt[:],
                                in_=w_out[:, k, n2 * 512 : (n2 + 1) * 512],
                            )
                            for mt in range(4):
                                nc.tensor.matmul(
                                    ps_l[mt][:],
                                    ym_sb[:, k, mt * P : (mt + 1) * P],
                                    wo_kt[:],
                                    start=(k == 0),
                                    stop=(k == 15),
                                )
                        for mt in range(4):
                            nc.vector.tensor_tensor(
                                x1_sb[:, mt, n2 * 512 : (n2 + 1) * 512],
                                ps_l[mt][:],
                                xmy_sb[:, mt, n2 * 512 : (n2 + 1) * 512],
                                op=ALU.add,
                            )

                # ======== PHASE 2: LN2 + router + MoE =======================
                with tc.tile_pool(name="p2", bufs=1) as p2:
                    xf_fm = p2.tile([P, 8, TPC], FP32, name="xf_fm")
                    wmat = p2.tile([P, 4, NEXP], FP32, name="wmat")
                    with tc.tile_pool(name="psT2", bufs=2, space="PSUM") as psT2, \
                         tc.tile_pool(name="psR", bufs=2, space="PSUM") as psR:
                        for mt in range(4):
                            xf_t = p2.tile(
                                [P, D], FP32, name="xf_t", tag="xf_t", bufs=2
                            )
                            _layernorm_tile(nc, p2, x1_sb[:, mt, :], s2b, b2b, xf_t, zero_col, eps_col)
                            for k in range(8):
                                tp = psT2.tile([P, P], FP32, name="tp2", tag="tp2")
                                nc.tensor.transpose(
                                    tp[:], xf_t[:, k * P : (k + 1) * P], ident[:]
                                )
                                if k % 2 == 0:
                                    nc.scalar.copy(
                                        xf_fm[:, k, mt * P : (mt + 1) * P], tp[:]
                                    )
                                else:
                                    nc.vector.tensor_copy(
                                        xf_fm[:, k, mt * P : (mt + 1) * P], tp[:]
                                    )
                        # router fp32
                        wrt_sb = p2.tile([P, 8, NEXP], FP32, name="wrt_sb")
                        nc.sync.dma_start(out=wrt_sb[:], in_=w_rt[:])
                        for mt in range(4):
                            psr = psR.tile([P, NEXP], FP32, name="psr", tag="psr")
                            for k in range(8):
                                nc.tensor.matmul(
                                    psr[:],
                                    xf_fm[:, k, mt * P : (mt + 1) * P],
                                    wrt_sb[:, k, :],
                                    start=(k == 0),
                                    stop=(k == 7),
                                )
                            pl = p2.tile([P, NEXP], FP32, name="pl", tag="pl", bufs=2)
                            nc.vector.tensor_copy(pl[:], psr[:])
                            nm1 = p2.tile([P, 1], FP32, name="nm1", tag="nm1", bufs=2)
                            nc.vector.tensor_reduce(
                                nm1[:],
                                pl[:],
                                axis=mybir.AxisListType.X,
                                op=ALU.max,
                                negate=True,
                            )
                            ep = p2.tile([P, NEXP], FP32, name="ep", tag="ep", bufs=2)
                            nc.scalar.activation(ep[:], pl[:], ACTF.Exp, bias=nm1[:])
                            eq = p2.tile([P, NEXP], FP32, name="eq", tag="eq", bufs=2)
                            nc.vector.tensor_scalar(
                                out=eq[:],
                                in0=ep[:],
                                scalar1=1.0,
                                scalar2=None,
                                op0=ALU.is_ge,
                            )
                            pm = p2.tile([P, NEXP], FP32, name="pm", tag="pm", bufs=2)
                            nc.vector.tensor_tensor(pm[:], ep[:], eq[:], op=ALU.subtract)
                            m2v = p2.tile([P, 1], FP32, name="m2v", tag="m2v", bufs=2)
                            nc.vector.tensor_reduce(
                                m2v[:], pm[:], axis=mybir.AxisListType.X, op=ALU.max
                            )
                            sel = p2.tile([P, NEXP], FP32, name="sel", tag="sel", bufs=2)
                            nc.vector.tensor_scalar(
                                out=sel[:],
                                in0=ep[:],
                                scalar1=m2v[:],
                                scalar2=None,
                                op0=ALU.is_ge,
                            )
                            den = p2.tile([P, 1], FP32, name="den", tag="den", bufs=2)
                            nc.vector.tensor_scalar(
                                out=den[:],
                                in0=m2v[:],
                                scalar1=1.0,
                                scalar2=None,
                                op0=ALU.add,
                            )
                            rcp = p2.tile([P, 1], FP32, name="rcp", tag="rcp", bufs=2)
                            nc.vector.reciprocal(rcp[:], den[:])
                            wm_t = p2.tile([P, NEXP], FP32, name="wm_t", tag="wm_t", bufs=2)
                            nc.vector.scalar_tensor_tensor(
                                out=wm_t[:],
                                in0=ep[:],
                                scalar=rcp[:],
                                in1=sel[:],
                                op0=ALU.mult,
                                op1=ALU.mult,
                            )
                            nc.vector.tensor_copy(wmat[:, mt, :], wm_t[:])

                    # bf16 copy of xf for experts
                    xf_bf = p2.tile([P, 8, TPC], BF16, name="xf_bf")
                    nc.vector.tensor_copy(
                        xf_bf[:].rearrange("p a b -> p (a b)"),
                        xf_fm[:].rearrange("p a b -> p (a b)"),
                    )

                    with tc.tile_pool(name="psE", bufs=2, space="PSUM") as psE, \
                         tc.tile_pool(name="psU", bufs=2, space="PSUM") as psU, \
                         tc.tile_pool(name="psD", bufs=2, space="PSUM") as psD:
                        for e in range(NEXP):
                            gu_sb = p2.tile(
                                [P, 16, TPC], BF16, name="gu_sb", tag="gu_sb", bufs=2
                            )
                            for m in range(16):
                                wg_m = p2.tile(
                                    [P, 8, P], BF16, name="wg_m", tag="wg_m", bufs=3
                                )
                                nc.sync.dma_start(
                                    out=wg_m[:], in_=wg[e, :, :, m * P : (m + 1) * P]
                                )
                                wu_m = p2.tile(
                                    [P, 8, P], BF16, name="wu_m", tag="wu_m", bufs=3
                                )
                                nc.sync.dma_start(
                                    out=wu_m[:], in_=wu[e, :, :, m * P : (m + 1) * P]
                                )
                                psg = psE.tile([P, TPC], FP32, name="psg", tag="psg")
                                psu = psU.tile([P, TPC], FP32, name="psu", tag="psu")
                                for k in range(8):
                                    nc.tensor.matmul(
                                        psg[:],
                                        wg_m[:, k, :],
                                        xf_bf[:, k, :],
                                        start=(k == 0),
                                        stop=(k == 7),
                                    )
                                for k in range(8):
                                    nc.tensor.matmul(
                                        psu[:],
                                        wu_m[:, k, :],
                                        xf_bf[:, k, :],
                                        start=(k == 0),
                                        stop=(k == 7),
                                    )
                                sg = p2.tile(
                                    [P, TPC], FP32, name="sg", tag="sg", bufs=2
                                )
                                nc.scalar.activation(sg[:], psg[:], ACTF.Silu, bias=zero_col[:])
                                nc.vector.tensor_tensor(
                                    gu_sb[:, m, :], sg[:], psu[:], op=ALU.mult
                                )
                            wd_e = p2.tile(
                                [P, 16, D], BF16, name="wd_e", tag="wd_e", bufs=1
                            )
                            nc.sync.dma_start(out=wd_e[:], in_=wd[e, :, :, :])
                            for mt in range(4):
                                for n2 in range(2):
                                    psd = psD.tile(
                                        [P, 512], FP32, name="psd", tag="psd"
                                    )
                                    for k2 in range(16):
                                        nc.tensor.matmul(
                                            psd[:],
                                            gu_sb[:, k2, mt * P : (mt + 1) * P],
                                            wd_e[:, k2, n2 * 512 : (n2 + 1) * 512],
                                            start=(k2 == 0),
                                            stop=(k2 == 15),
                                        )
                                    nc.vector.scalar_tensor_tensor(
                                        out=x1_sb[:, mt, n2 * 512 : (n2 + 1) * 512],
                                        in0=psd[:],
                                        scalar=wmat[:, mt, e : e + 1],
                                        in1=x1_sb[:, mt, n2 * 512 : (n2 + 1) * 512],
                                        op0=ALU.mult,
                                        op1=ALU.add,
                                    )
                # write output
                nc.sync.dma_start(
                    out=out_p.rearrange("(a p) d -> p a d", p=P), in_=x1_sb[:]
                )
    nc.compile()
    return nc


_NC_CACHE = None


def _get_nc():
    global _NC_CACHE
    if _NC_CACHE is None:
        _NC_CACHE = build()
    return _NC_CACHE


def _r(a, kp, *dims):
    """reshape [kp*P, ...] -> [P, kp, ...] weight layout for lhsT k-tiles."""
    a = np.ascontiguousarray(a)
    s = a.shape
    return np.ascontiguousarray(
        a.reshape(kp, P, *s[1:]).transpose(1, 0, *range(2, a.ndim + 1))
    )


def prep_in_maps(
    x, ssm_state, conv_state, ln1_s, ln1_b, ln2_s, ln2_b, in_proj_w, conv_w, conv_b,
    x_proj_w, dt_proj_w, dt_proj_b, A_log, Dp, out_proj_w, router_w, w_gate, w_up,
    w_down,
):
    f32 = np.float32
    bf16 = ml_dtypes.bfloat16
    x2 = np.asarray(x, f32).reshape(NTOK, D)
    w_in_z = _r(np.asarray(in_proj_w, f32)[DI:, :].T, 8)  # [128, 8, 2048]
    w_out = _r(np.asarray(out_proj_w, f32).T, 16)  # [128, 16, 1024]
    w_rt = _r(np.asarray(router_w, f32).T, 8)  # [128, 8, 8]
    wg_a = np.stack(
        [_r(np.asarray(w_gate[e], f32).T, 8).astype(bf16) for e in range(NEXP)]
    )
    wu_a = np.stack(
        [_r(np.asarray(w_up[e], f32).T, 8).astype(bf16) for e in range(NEXP)]
    )
    wd_a = np.stack(
        [_r(np.asarray(w_down[e], f32).T, 16).astype(bf16) for e in range(NEXP)]
    )
    ln = {
        "ln1_s": np.asarray(ln1_s, f32).reshape(1, D),
        "ln1_b": np.asarray(ln1_b, f32).reshape(1, D),
        "ln2_s": np.asarray(ln2_s, f32).reshape(1, D),
        "ln2_b": np.asarray(ln2_b, f32).reshape(1, D),
    }
    in_maps = []
    for c in range(NCORE):
        ch0, ch1 = c * CH, (c + 1) * CH
        tk0, tk1 = c * TPC, (c + 1) * TPC
        m = {
            "x_full": x2,
            "x_my": np.ascontiguousarray(x2[tk0:tk1]),
            "w_in_x": _r(np.asarray(in_proj_w, f32)[ch0:ch1, :].T, 8),
            "w_in_z": w_in_z,
            "w_xp": _r(np.asarray(x_proj_w, f32)[:, ch0:ch1].T, MT),
            "w_dt": np.ascontiguousarray(np.asarray(dt_proj_w, f32)[ch0:ch1, :].T),
            "w_out": w_out,
            "w_rt": w_rt,
            "wg": wg_a,
            "wu": wu_a,
            "wd": wd_a,
            "conv_w": _r(np.asarray(conv_w, f32)[ch0:ch1, 0, :], MT),
            "conv_b": _r(np.asarray(conv_b, f32)[ch0:ch1], MT),
            "dt_bias": _r(np.asarray(dt_proj_b, f32)[ch0:ch1], MT),
            "dp": _r(np.asarray(Dp, f32)[ch0:ch1], MT),
            "a_log": _r(np.asarray(A_log, f32)[ch0:ch1, :], MT),
            "ssm0": np.ascontiguousarray(
                np.asarray(ssm_state, f32)[:, ch0:ch1, :].reshape(Bb, MT, P, DS)
            ),
            "conv0": np.ascontiguousarray(
                np.asarray(conv_state, f32)[:, ch0:ch1, :].reshape(Bb, MT, P, 3)
            ),
            **ln,
        }
        in_maps.append(m)
    return in_maps


LAST_EXEC_NS = None


def kernel(**inputs):
    global LAST_EXEC_NS
    import os

    nc = _get_nc()
    in_maps = prep_in_maps(**inputs)
    kw = {}
    if os.environ.get("KERNEL_TRACE"):
        kw["trace"] = True
    res = run_bass_kernel_spmd(nc, in_maps, core_ids=list(range(NCORE)), **kw)
    LAST_EXEC_NS = getattr(res, "exec_time_ns", None)
    out = np.concatenate([res.results[c]["out"] for c in range(NCORE)], axis=0)
    return out.reshape(Bb, T, D).astype(np.float32)


if __name__ == "__main__":
    print("building...")
    nc = _get_nc()
    print("build ok")


# revision 10
# speedup vs baseline: 1.1127x; 1.1127x over previous
"""JambaBlock Trainium2 kernel: 8-core SPMD bass/Tile implementation.

Sharding:
  - mamba in_proj_x/conv/scan: channel-sharded (256 of 2048 d_inner per core)
  - x_proj partials -> AllReduce [96, 4096] (1.6 MB)
  - scan y -> AllToAll [8, 256, 512] to token-shard (4.2 MB)
  - z_proj / out_proj / LN2 / router / MoE: token-sharded (512 of 4096 per core)
  - MoE: dense all-8-experts in bf16, fp32 router, weighted accumulate
  - output: per-core [512, 1024] slice, host concat
"""
import sys

sys.path.insert(0, "/opt/trn_rl_repo")

import numpy as np
import ml_dtypes

import concourse.bass as bass
import concourse.bacc as bacc
import concourse.mybir as mybir
import concourse.tile as tile
from concourse.bass_utils import run_bass_kernel_spmd
from concourse.masks import make_identity

FP32 = mybir.dt.float32
BF16 = mybir.dt.bfloat16
ALU = mybir.AluOpType
ACTF = mybir.ActivationFunctionType

NCORE = 8
D = 1024
DI = 2048
CH = DI // NCORE  # 256 channels per core
DS = 16
DTR = 64
Bb = 2
T = 2048
NTOK = Bb * T  # 4096
TPC = NTOK // NCORE  # 512 tokens per core
HID = 2048
NEXP = 8
EPS = 1e-5
P = 128
MT = CH // P  # 2 channel tiles per core
TCH = 1024  # scan time-chunk
NCH = T // TCH  # 2 chunks per batch seq


def _bcast_rows(nc, dst, src_row):
    """Replicate src_row [1, W] across dst [128, W] via doubling DMAs."""
    nc.sync.dma_start(out=dst[0:1, :], in_=src_row)
    p = 1
    while p < P:
        nc.sync.dma_start(out=dst[p : 2 * p, :], in_=dst[0:p, :])
        p *= 2


def _layernorm_tile(nc, pool, xt, s_bc, b_bc, out_t, zc, ec):
    """LN over free dim of xt [128, D] -> out_t (fp32). s_bc/b_bc [128, D]."""
    s1 = pool.tile([P, 1], FP32, name="ln_s1", tag="ln_s1", bufs=2)
    nm = pool.tile([P, 1], FP32, name="ln_nm", tag="ln_nm", bufs=2)
    nc.vector.tensor_reduce(s1[:], xt[:], axis=mybir.AxisListType.X, op=ALU.add)
    nc.vector.tensor_scalar(
        out=nm[:], in0=s1[:], scalar1=-1.0 / D, scalar2=None, op0=ALU.mult
    )
    xc = pool.tile([P, D], FP32, name="ln_xc", tag="ln_xc", bufs=2)
    nc.scalar.activation(xc[:], xt[:], ACTF.Identity, bias=nm[:])
    sq = pool.tile([P, D], FP32, name="ln_sq", tag="ln_sq", bufs=2)
    ssq = pool.tile([P, 1], FP32, name="ln_ssq", tag="ln_ssq", bufs=2)
    nc.scalar.activation(sq[:], xc[:], ACTF.Square, bias=zc[:], accum_out=ssq[:])
    sd = pool.tile([P, 1], FP32, name="ln_sd", tag="ln_sd", bufs=2)
    nc.scalar.activation(sd[:], ssq[:], ACTF.Sqrt, scale=1.0 / D, bias=ec[:])
    r = pool.tile([P, 1], FP32, name="ln_r", tag="ln_r", bufs=2)
    nc.vector.reciprocal(r[:], sd[:])
    t2 = pool.tile([P, D], FP32, name="ln_t2", tag="ln_t2", bufs=2)
    nc.vector.scalar_tensor_tensor(
        out=t2[:], in0=xc[:], scalar=r[:], in1=s_bc[:], op0=ALU.mult, op1=ALU.mult
    )
    nc.vector.tensor_tensor(out_t[:], t2[:], b_bc[:], op=ALU.add)


def build():
    nc = bacc.Bacc(None, target_bir_lowering=False, debug=False)

    # ---- I/O -------------------------------------------------------------
    x_full = nc.declare_dram_parameter("x_full", [NTOK, D], FP32, isOutput=False)
    x_my = nc.declare_dram_parameter("x_my", [TPC, D], FP32, isOutput=False)
    w_in_x = nc.declare_dram_parameter("w_in_x", [P, 8, CH], FP32, isOutput=False)
    w_in_z = nc.declare_dram_parameter("w_in_z", [P, 8, DI], FP32, isOutput=False)
    w_xp = nc.declare_dram_parameter("w_xp", [P, MT, 96], FP32, isOutput=False)
    w_dt = nc.declare_dram_parameter("w_dt", [DTR, CH], FP32, isOutput=False)
    w_out = nc.declare_dram_parameter("w_out", [P, 16, D], FP32, isOutput=False)
    w_rt = nc.declare_dram_parameter("w_rt", [P, 8, NEXP], FP32, isOutput=False)
    wg = nc.declare_dram_parameter("wg", [NEXP, P, 8, HID], BF16, isOutput=False)
    wu = nc.declare_dram_parameter("wu", [NEXP, P, 8, HID], BF16, isOutput=False)
    wd = nc.declare_dram_parameter("wd", [NEXP, P, 16, D], BF16, isOutput=False)
    conv_w = nc.declare_dram_parameter("conv_w", [P, MT, 4], FP32, isOutput=False)
    conv_b = nc.declare_dram_parameter("conv_b", [P, MT], FP32, isOutput=False)
    dt_bias = nc.declare_dram_parameter("dt_bias", [P, MT], FP32, isOutput=False)
    dp_in = nc.declare_dram_parameter("dp", [P, MT], FP32, isOutput=False)
    a_log = nc.declare_dram_parameter("a_log", [P, MT, DS], FP32, isOutput=False)
    ssm0 = nc.declare_dram_parameter("ssm0", [Bb, MT, P, DS], FP32, isOutput=False)
    conv0 = nc.declare_dram_parameter("conv0", [Bb, MT, P, 3], FP32, isOutput=False)
    ln1_s = nc.declare_dram_parameter("ln1_s", [1, D], FP32, isOutput=False)
    ln1_b = nc.declare_dram_parameter("ln1_b", [1, D], FP32, isOutput=False)
    ln2_s = nc.declare_dram_parameter("ln2_s", [1, D], FP32, isOutput=False)
    ln2_b = nc.declare_dram_parameter("ln2_b", [1, D], FP32, isOutput=False)
    out_p = nc.declare_dram_parameter("out", [TPC, D], FP32, isOutput=True)

    rg = [list(range(NCORE))]

    with tile.TileContext(nc) as tc:
        with tc.tile_pool(name="dram", bufs=1, space="DRAM") as dpool:
            xcs_d = dpool.tile([MT, P, NTOK], FP32, name="xcs_d")
            xz_d = dpool.tile([MT, P, NTOK], FP32, name="xz_d")
            z_d = dpool.tile([16, P, TPC], FP32, name="z_d")
            dt_d = dpool.tile([MT, P, NTOK], FP32, name="dt_d")
            ar_in = dpool.tile([96, NTOK], FP32, name="ar_in")
            ar_out = dpool.tile([96, NTOK], FP32, name="ar_out", addr_space="Shared")
            a2a_in = dpool.tile([NCORE, CH, TPC], FP32, name="a2a_in")
            a2a_out = dpool.tile([NCORE, CH, TPC], FP32, name="a2a_out")

            with tc.tile_pool(name="per", bufs=1) as per:
                # ---- persistent small tensors -----------------------------
                ident = per.tile([P, P], FP32, name="ident")
                make_identity(nc, ident)
                zero_col = per.tile([P, 1], FP32, name="zero_col")
                nc.vector.memset(zero_col[:], 0.0)
                eps_col = per.tile([P, 1], FP32, name="eps_col")
                nc.vector.memset(eps_col[:], EPS)
                one_col = per.tile([P, 1], FP32, name="one_col")
                nc.vector.memset(one_col[:], 1.0)
                ones_row = per.tile([1, P], FP32, name="ones_row")
                nc.vector.memset(ones_row[:], 1.0)
                s1b = per.tile([P, D], FP32, name="s1b")
                b1b = per.tile([P, D], FP32, name="b1b")
                s2b = per.tile([P, D], FP32, name="s2b")
                b2b = per.tile([P, D], FP32, name="b2b")
                _bcast_rows(nc, s1b, ln1_s[:])
                _bcast_rows(nc, b1b, ln1_b[:])
                _bcast_rows(nc, s2b, ln2_s[:])
                _bcast_rows(nc, b2b, ln2_b[:])
                cw_sb = per.tile([P, MT, 4], FP32, name="cw_sb")
                nc.sync.dma_start(out=cw_sb[:], in_=conv_w[:])
                cb_sb = per.tile([P, MT], FP32, name="cb_sb")
                nc.sync.dma_start(out=cb_sb[:], in_=conv_b[:])
                dtb_sb = per.tile([P, MT], FP32, name="dtb_sb")
                nc.sync.dma_start(out=dtb_sb[:], in_=dt_bias[:])
                dp_sb = per.tile([P, MT], FP32, name="dp_sb")
                nc.sync.dma_start(out=dp_sb[:], in_=dp_in[:])
                al_sb = per.tile([P, MT, DS], FP32, name="al_sb")
                nc.sync.dma_start(out=al_sb[:], in_=a_log[:])
                a_sb = per.tile([P, MT, DS], FP32, name="a_sb")
                # A = -exp(A_log)
                nc.scalar.activation(
                    a_sb[:, :, :].rearrange("p a b -> p (a b)"),
                    al_sb[:, :, :].rearrange("p a b -> p (a b)"),
                    ACTF.Exp,
                    bias=zero_col[:],
                )
                nc.vector.tensor_scalar(
                    out=a_sb[:, :, :].rearrange("p a b -> p (a b)"),
                    in0=a_sb[:, :, :].rearrange("p a b -> p (a b)"),
                    scalar1=-1.0,
                    scalar2=None,
                    op0=ALU.mult,
                )
                w_xp_sb = per.tile([P, MT, 96], FP32, name="w_xp_sb")
                nc.sync.dma_start(out=w_xp_sb[:], in_=w_xp[:])
                w_dt_sb = per.tile([DTR, CH], FP32, name="w_dt_sb")
                nc.sync.dma_start(out=w_dt_sb[:], in_=w_dt[:])
                xmy_sb = per.tile([P, 4, D], FP32, name="xmy_sb")
                nc.sync.dma_start(
                    out=xmy_sb[:],
                    in_=x_my.rearrange("(a p) d -> p a d", p=P),
                )
                # proj (x_proj output after AllReduce) [96, NTOK]
                proj_sb = per.tile([96, NTOK], FP32, name="proj_sb")
                # x1 (also the final accumulator), token-major
                x1_sb = per.tile([P, 4, D], FP32, name="x1_sb")

                # ======== PHASE 1a: LN1 + transposes + in_proj_x + z_proj ===
                with tc.tile_pool(name="p1", bufs=1) as p1, tc.tile_pool(
                    name="psT", bufs=2, space="PSUM"
                ) as psT, tc.tile_pool(name="psA", bufs=2, space="PSUM") as psA:
                    w_in_x_sb = p1.tile([P, 8, CH], FP32, name="w_in_x_sb")
                    nc.sync.dma_start(out=w_in_x_sb[:], in_=w_in_x[:])

                    for ns in range(8):  # 512-token slices
                        xn_fm = p1.tile(
                            [P, 8, 512], FP32, name="xn_fm", tag="xn_fm", bufs=2
                        )
                        for tt in range(4):
                            row0 = (ns * 4 + tt) * P
                            xt = p1.tile([P, D], FP32, name="xt", tag="xt", bufs=3)
                            nc.sync.dma_start(
                                out=xt[:], in_=x_full[row0 : row0 + P, :]
                            )
                            xn_t = p1.tile(
                                [P, D], FP32, name="xn_t", tag="xn_t", bufs=2
                            )
                            _layernorm_tile(nc, p1, xt, s1b, b1b, xn_t, zero_col, eps_col)
                            for k in range(8):
                                tp = psT.tile([P, P], FP32, name="tp", tag="tp")
                                nc.tensor.transpose(
                                    tp[:], xn_t[:, k * P : (k + 1) * P], ident[:]
                                )
                                eng = nc.scalar if (k % 2 == 0) else nc.vector
                                if eng is nc.scalar:
                                    nc.scalar.copy(
                                        xn_fm[:, k, tt * P : (tt + 1) * P], tp[:]
                                    )
                                else:
                                    nc.vector.tensor_copy(
                                        xn_fm[:, k, tt * P : (tt + 1) * P], tp[:]
                                    )
                        # in_proj_x for this token slice
                        for m in range(MT):
                            ps = psA.tile([P, 512], FP32, name="ps_inx", tag="ps_inx")
                            for k in range(8):
                                nc.tensor.matmul(
                                    ps[:],
                                    w_in_x_sb[:, k, m * P : (m + 1) * P],
                                    xn_fm[:, k, :],
                                    start=(k == 0),
                                    stop=(k == 7),
                                )
                            xz_t = p1.tile(
                                [P, 512], FP32, name="xz_t", tag="xz_t", bufs=2
                            )
                            nc.scalar.copy(xz_t[:], ps[:])
                            nc.sync.dma_start(
                                out=xz_d[m, :, ns * 512 : (ns + 1) * 512], in_=xz_t[:]
                            )

                    # z_proj: token slice of this core only
                    xnz_fm = p1.tile([P, 8, 512], FP32, name="xnz_fm")
                    for tt in range(4):
                        xn_t = p1.tile([P, D], FP32, name="xn_t", tag="xn_t", bufs=2)
                        _layernorm_tile(nc, p1, xmy_sb[:, tt, :], s1b, b1b, xn_t, zero_col, eps_col)
                        for k in range(8):
                            tp = psT.tile([P, P], FP32, name="tp", tag="tp")
                            nc.tensor.transpose(
                                tp[:], xn_t[:, k * P : (k + 1) * P], ident[:]
                            )
                            if k % 2 == 0:
                                nc.scalar.copy(
                                    xnz_fm[:, k, tt * P : (tt + 1) * P], tp[:]
                                )
                            else:
                                nc.vector.tensor_copy(
                                    xnz_fm[:, k, tt * P : (tt + 1) * P], tp[:]
                                )
                    for m2 in range(16):
                        wz_m = p1.tile(
                            [P, 8, P], FP32, name="wz_m", tag="wz_m", bufs=3
                        )
                        nc.sync.dma_start(
                            out=wz_m[:], in_=w_in_z[:, :, m2 * P : (m2 + 1) * P]
                        )
                        ps = psA.tile([P, 512], FP32, name="ps_z", tag="ps_z")
                        for k in range(8):
                            nc.tensor.matmul(
                                ps[:],
                                wz_m[:, k, :],
                                xnz_fm[:, k, :],
                                start=(k == 0),
                                stop=(k == 7),
                            )
                        zt = p1.tile([P, TPC], FP32, name="zt", tag="zt", bufs=2)
                        nc.scalar.activation(zt[:], ps[:], ACTF.Silu, bias=zero_col[:])
                        nc.sync.dma_start(out=z_d[m2, :, :], in_=zt[:])

                # ---- conv (depthwise causal, taps=4) + SiLU -> xcs_d ---
                with tc.tile_pool(name="p1b", bufs=1) as p1, tc.tile_pool(
                    name="psB", bufs=2, space="PSUM"
                ) as psA:
                    for b in range(Bb):
                        for m in range(MT):
                            xpad = p1.tile(
                                [P, T + 3], FP32, name="xpad", tag="xpad", bufs=2
                            )
                            nc.sync.dma_start(
                                out=xpad[:, 0:3], in_=conv0[b, m, :, :]
                            )
                            nc.sync.dma_start(
                                out=xpad[:, 3:],
                                in_=xz_d[m, :, b * T : (b + 1) * T],
                            )
                            cacc = p1.tile(
                                [P, T], FP32, name="cacc", tag="cacc", bufs=2
                            )
                            nc.vector.tensor_scalar(
                                out=cacc[:],
                                in0=xpad[:, 0:T],
                                scalar1=cw_sb[:, m, 0:1],
                                scalar2=None,
                                op0=ALU.mult,
                            )
                            for k in range(1, 4):
                                nc.vector.scalar_tensor_tensor(
                                    out=cacc[:],
                                    in0=xpad[:, k : k + T],
                                    scalar=cw_sb[:, m, k : k + 1],
                                    in1=cacc[:],
                                    op0=ALU.mult,
                                    op1=ALU.add,
                                )
                            xcs_t = p1.tile(
                                [P, T], FP32, name="xcs_t", tag="xcs_t", bufs=2
                            )
                            nc.scalar.activation(
                                xcs_t[:], cacc[:], ACTF.Silu, bias=cb_sb[:, m : m + 1]
                            )
                            nc.sync.dma_start(
                                out=xcs_d[m, :, b * T : (b + 1) * T], in_=xcs_t[:]
                            )

                    # ---- x_proj partials + AllReduce -----------------------
                    for n in range(8):
                        ps = psA.tile([96, 512], FP32, name="ps_xp", tag="ps_xp")
                        for k in range(MT):
                            xc_kn = p1.tile(
                                [P, 512], FP32, name="xc_kn", tag="xc_kn", bufs=3
                            )
                            nc.sync.dma_start(
                                out=xc_kn[:],
                                in_=xcs_d[k, :, n * 512 : (n + 1) * 512],
                            )
                            nc.tensor.matmul(
                                ps[:],
                                w_xp_sb[:, k, :],
                                xc_kn[:],
                                start=(k == 0),
                                stop=(k == MT - 1),
                            )
                        pp_t = p1.tile([96, 512], FP32, name="pp_t", tag="pp_t", bufs=2)
                        nc.vector.tensor_copy(pp_t[:], ps[:])
                        nc.sync.dma_start(
                            out=ar_in[:, n * 512 : (n + 1) * 512], in_=pp_t[:]
                        )
                    nc.gpsimd.collective_compute(
                        "AllReduce",
                        ALU.add,
                        replica_groups=rg,
                        ins=[ar_in.opt()],
                        outs=[ar_out.opt()],
                    )
                    nc.sync.dma_start(out=proj_sb[:], in_=ar_out[:])

                    # ---- dt = softplus(dtraw @ w_dt + b) -------------------
                    for m in range(MT):
                        for n in range(8):
                            ps = psA.tile([P, 512], FP32, name="ps_dt", tag="ps_dt")
                            nc.tensor.matmul(
                                ps[:],
                                w_dt_sb[:, m * P : (m + 1) * P],
                                proj_sb[0:DTR, n * 512 : (n + 1) * 512],
                                start=True,
                                stop=True,
                            )
                            et = p1.tile([P, 512], FP32, name="et", tag="et", bufs=2)
                            nc.scalar.activation(
                                et[:], ps[:], ACTF.Exp, bias=dtb_sb[:, m : m + 1]
                            )
                            dt_t = p1.tile(
                                [P, 512], FP32, name="dt_t", tag="dt_t", bufs=2
                            )
                            nc.scalar.activation(dt_t[:], et[:], ACTF.Ln, bias=one_col[:])
                            nc.sync.dma_start(
                                out=dt_d[m, :, n * 512 : (n + 1) * 512], in_=dt_t[:]
                            )

                # ======== PHASE 1b: the selective scan ======================
                with tc.tile_pool(name="sc", bufs=1) as sc, tc.tile_pool(
                    name="psSC", bufs=2, space="PSUM"
                ) as psSC:
                    for b in range(Bb):
                        dt_u = []
                        u_u = []
                        y_acc = []
                        hstate = []
                        for m in range(MT):
                            dtt = sc.tile(
                                [P, T], FP32, name=f"dt_u{m}", tag=f"dt_u{m}", bufs=1
                            )
                            nc.sync.dma_start(
                                out=dtt[:], in_=dt_d[m, :, b * T : (b + 1) * T]
                            )
                            dt_u.append(dtt)
                            xcst = sc.tile(
                                [P, T], FP32, name=f"xc_u{m}", tag=f"xc_u{m}", bufs=1
                            )
                            nc.sync.dma_start(
                                out=xcst[:], in_=xcs_d[m, :, b * T : (b + 1) * T]
                            )
                            ut = sc.tile(
                                [P, T], FP32, name=f"u_u{m}", tag=f"u_u{m}", bufs=1
                            )
                            nc.vector.tensor_tensor(ut[:], dtt[:], xcst[:], op=ALU.mult)
                            u_u.append(ut)
                            hs = sc.tile(
                                [P, DS], FP32, name=f"hs{m}", tag=f"hs{m}", bufs=2
                            )
                            nc.sync.dma_start(out=hs[:], in_=ssm0[b, m, :, :])
                            hstate.append(hs)
                            ya = sc.tile(
                                [P, T], FP32, name=f"ya{m}", tag=f"ya{m}", bufs=1
                            )
                            y_acc.append(ya)

                        for c in range(NCH):
                            t0 = c * TCH
                            for ds in range(DS):
                                brow = sc.tile(
                                    [1, TCH], FP32, name="brow", tag="brow", bufs=3
                                )
                                nc.sync.dma_start(
                                    out=brow[:],
                                    in_=ar_out[
                                        64 + ds : 65 + ds,
                                        b * T + t0 : b * T + t0 + TCH,
                                    ],
                                )
                                crow = sc.tile(
                                    [1, TCH], FP32, name="crow", tag="crow", bufs=3
                                )
                                nc.sync.dma_start(
                                    out=crow[:],
                                    in_=ar_out[
                                        80 + ds : 81 + ds,
                                        b * T + t0 : b * T + t0 + TCH,
                                    ],
                                )
                                ps_bb = psSC.tile(
                                    [P, TCH], FP32, name="ps_bb", tag="ps_bb"
                                )
                                ps_cc = psSC.tile(
                                    [P, TCH], FP32, name="ps_cc", tag="ps_cc"
                                )
                                for n2 in range(TCH // 512):
                                    nc.tensor.matmul(
                                        ps_bb[:, n2 * 512 : (n2 + 1) * 512],
                                        ones_row[:],
                                        brow[0:1, n2 * 512 : (n2 + 1) * 512],
                                        start=True,
                                        stop=True,
                                    )
                                    nc.tensor.matmul(
                                        ps_cc[:, n2 * 512 : (n2 + 1) * 512],
                                        ones_row[:],
                                        crow[0:1, n2 * 512 : (n2 + 1) * 512],
                                        start=True,
                                        stop=True,
                                    )
                                cbt = sc.tile(
                                    [P, TCH], FP32, name="cbt", tag="cbt", bufs=2
                                )
                                nc.scalar.copy(cbt[:], ps_cc[:])
                                for m in range(MT):
                                    dA = sc.tile(
                                        [P, TCH], FP32, name="dA", tag="dA", bufs=2
                                    )
                                    nc.scalar.activation(
                                        dA[:],
                                        dt_u[m][:, t0 : t0 + TCH],
                                        ACTF.Exp,
                                        bias=zero_col[:],
                                        scale=a_sb[:, m, ds : ds + 1],
                                    )
                                    dBx = sc.tile(
                                        [P, TCH], FP32, name="dBx", tag="dBx", bufs=2
                                    )
                                    nc.vector.tensor_tensor(
                                        dBx[:],
                                        u_u[m][:, t0 : t0 + TCH],
                                        ps_bb[:],
                                        op=ALU.mult,
                                    )
                                    h = sc.tile(
                                        [P, TCH], FP32, name="h", tag="h", bufs=2
                                    )
                                    nc.vector.tensor_tensor_scan(
                                        h[:],
                                        dA[:],
                                        dBx[:],
                                        hstate[m][:, ds : ds + 1],
                                        op0=ALU.mult,
                                        op1=ALU.add,
                                    )
                                    # save final state for chunk chaining
                                    nc.vector.tensor_copy(
                                        hstate[m][:, ds : ds + 1], h[:, TCH - 1 : TCH]
                                    )
                                    if ds == 0:
                                        nc.vector.tensor_tensor(
                                            y_acc[m][:, t0 : t0 + TCH],
                                            h[:],
                                            cbt[:],
                                            op=ALU.mult,
                                        )
                                    else:
                                        hC = sc.tile(
                                            [P, TCH], FP32, name="hC", tag="hC", bufs=2
                                        )
                                        nc.gpsimd.tensor_tensor(
                                            hC[:], h[:], cbt[:], op=ALU.mult
                                        )
                                        nc.vector.tensor_tensor(
                                            y_acc[m][:, t0 : t0 + TCH],
                                            y_acc[m][:, t0 : t0 + TCH],
                                            hC[:],
                                            op=ALU.add,
                                        )
                        # add Dp * xcs, then ship chunks to a2a_in
                        for m in range(MT):
                            xcst = sc.tile(
                                [P, T], FP32, name=f"xc2_{m}", tag=f"xc_u{m}", bufs=1
                            )
                            nc.sync.dma_start(
                                out=xcst[:], in_=xcs_d[m, :, b * T : (b + 1) * T]
                            )
                            nc.vector.scalar_tensor_tensor(
                                out=y_acc[m][:],
                                in0=xcst[:],
                                scalar=dp_sb[:, m : m + 1],
                                in1=y_acc[m][:],
                                op0=ALU.mult,
                                op1=ALU.add,
                            )
                            for tc4 in range(4):
                                j = b * 4 + tc4
                                nc.sync.dma_start(
                                    out=a2a_in[j, m * P : (m + 1) * P, :],
                                    in_=y_acc[m][:, tc4 * TPC : (tc4 + 1) * TPC],
                                )

                nc.gpsimd.collective_compute(
                    "AllToAll",
                    ALU.bypass,
                    replica_groups=rg,
                    ins=[a2a_in.opt()],
                    outs=[a2a_out.opt()],
                )

                # ======== PHASE 1c: ymul + out_proj + residual ==============
                with tc.tile_pool(name="op", bufs=1) as op, tc.tile_pool(
                    name="psOP", bufs=1, space="PSUM"
                ) as psOP:
                    ym_sb = op.tile([P, 16, TPC], FP32, name="ym_sb")
                    for r in range(NCORE):
                        nc.sync.dma_start(
                            out=ym_sb[:, 2 * r, :], in_=a2a_out[r, 0:P, :]
                        )
                        nc.sync.dma_start(
                            out=ym_sb[:, 2 * r + 1, :], in_=a2a_out[r, P:CH, :]
                        )
                    # ymul = y * silu(z)
                    for kz in range(16):
                        zt2 = op.tile([P, TPC], FP32, name="zt2", tag="zt2", bufs=3)
                        nc.sync.dma_start(out=zt2[:], in_=z_d[kz, :, :])
                        nc.vector.tensor_tensor(
                            ym_sb[:, kz, :], ym_sb[:, kz, :], zt2[:], op=ALU.mult
                        )
                    for n2 in range(2):
                        ps_l = [
                            psOP.tile([P, 512], FP32, name=f"psop{mt}", tag=f"psop{mt}")
                            for mt in range(4)
                        ]
                        for k in range(16):
                            wo_kt = op.tile(
                                [P, 512], FP32, name="wo_kt", tag="wo_kt", bufs=3
                            )
                            nc.sync.dma_start(
                                out=wo_kt[:],
                                in_=w_out[:, k, n2 * 512 : (n2 + 1) * 512],
                            )
                            for mt in range(4):
                                nc.tensor.matmul(
                                    ps_l[mt][:],
                                    ym_sb[:, k, mt * P : (mt + 1) * P],
                                    wo_kt[:],
                                    start=(k == 0),
                                    stop=(k == 15),
                                )
                        for mt in range(4):
                            nc.vector.tensor_tensor(
                                x1_sb[:, mt, n2 * 512 : (n2 + 1) * 512],
                                ps_l[mt][:],
                                xmy_sb[:, mt, n2 * 512 : (n2 + 1) * 512],
                                op=ALU.add,
                            )

                # ======== PHASE 2: LN2 + router + MoE =======================
                with tc.tile_pool(name="p2", bufs=1) as p2:
                    xf_fm = p2.tile([P, 8, TPC], FP32, name="xf_fm")
                    wmat = p2.tile([P, 4, NEXP], FP32, name="wmat")
                    with tc.tile_pool(name="psT2", bufs=2, space="PSUM") as psT2, \
                         tc.tile_pool(name="psR", bufs=2, space="PSUM") as psR:
                        for mt in range(4):
                            xf_t = p2.tile(
                                [P, D], FP32, name="xf_t", tag="xf_t", bufs=2
                            )
                            _layernorm_tile(nc, p2, x1_sb[:, mt, :], s2b, b2b, xf_t, zero_col, eps_col)
                            for k in range(8):
                                tp = psT2.tile([P, P], FP32, name="tp2", tag="tp2")
                                nc.tensor.transpose(
                                    tp[:], xf_t[:, k * P : (k + 1) * P], ident[:]
                                )
                                if k % 2 == 0:
                                    nc.scalar.copy(
                                        xf_fm[:, k, mt * P : (mt + 1) * P], tp[:]
                                    )
                                else:
                                    nc.vector.tensor_copy(
                                        xf_fm[:, k, mt * P : (mt + 1) * P], tp[:]
                                    )
                        # router fp32
                        wrt_sb = p2.tile([P, 8, NEXP], FP32, name="wrt_sb")
                        nc.sync.dma_start(out=wrt_sb[:], in_=w_rt[:])
                        for mt in range(4):
                            psr = psR.tile([P, NEXP], FP32, name="psr", tag="psr")
                            for k in range(8):
                                nc.tensor.matmul(
                                    psr[:],
                                    xf_fm[:, k, mt * P : (mt + 1) * P],
                                    wrt_sb[:, k, :],
                                    start=(k == 0),
                                    stop=(k == 7),
                                )
                            pl = p2.tile([P, NEXP], FP32, name="pl", tag="pl", bufs=2)
                            nc.vector.tensor_copy(pl[:], psr[:])
                            nm1 = p2.tile([P, 1], FP32, name="nm1", tag="nm1", bufs=2)
                            nc.vector.tensor_reduce(
                                nm1[:],
                                pl[:],
                                axis=mybir.AxisListType.X,
                                op=ALU.max,
                                negate=True,
                            )
                            ep = p2.tile([P, NEXP], FP32, name="ep", tag="ep", bufs=2)
                            nc.scalar.activation(ep[:], pl[:], ACTF.Exp, bias=nm1[:])
                            eq = p2.tile([P, NEXP], FP32, name="eq", tag="eq", bufs=2)
                            nc.vector.tensor_scalar(
                                out=eq[:],
                                in0=ep[:],
                                scalar1=1.0,
                                scalar2=None,
                                op0=ALU.is_ge,
                            )
                            pm = p2.tile([P, NEXP], FP32, name="pm", tag="pm", bufs=2)
                            nc.vector.tensor_tensor(pm[:], ep[:], eq[:], op=ALU.subtract)
                            m2v = p2.tile([P, 1], FP32, name="m2v", tag="m2v", bufs=2)
                            nc.vector.tensor_reduce(
                                m2v[:], pm[:], axis=mybir.AxisListType.X, op=ALU.max
                            )
                            sel = p2.tile([P, NEXP], FP32, name="sel", tag="sel", bufs=2)
                            nc.vector.tensor_scalar(
                                out=sel[:],
                                in0=ep[:],
                                scalar1=m2v[:],
                                scalar2=None,
                                op0=ALU.is_ge,
                            )
                            den = p2.tile([P, 1], FP32, name="den", tag="den", bufs=2)
                            nc.vector.tensor_scalar(
                                out=den[:],
                                in0=m2v[:],
                                scalar1=1.0,
                                scalar2=None,
                                op0=ALU.add,
                            )
                            rcp = p2.tile([P, 1], FP32, name="rcp", tag="rcp", bufs=2)
                            nc.vector.reciprocal(rcp[:], den[:])
                            wm_t = p2.tile([P, NEXP], FP32, name="wm_t", tag="wm_t", bufs=2)
                            nc.vector.scalar_tensor_tensor(
                                out=wm_t[:],
                                in0=ep[:],
                                scalar=rcp[:],
                                in1=sel[:],
                                op0=ALU.mult,
                                op1=ALU.mult,
                            )
                            nc.vector.tensor_copy(wmat[:, mt, :], wm_t[:])

                    # bf16 copy of xf for experts
                    xf_bf = p2.tile([P, 8, TPC], BF16, name="xf_bf")
                    nc.vector.tensor_copy(
                        xf_bf[:].rearrange("p a b -> p (a b)"),
                        xf_fm[:].rearrange("p a b -> p (a b)"),
                    )

                    with tc.tile_pool(name="psE", bufs=2, space="PSUM") as psE, \
                         tc.tile_pool(name="psU", bufs=2, space="PSUM") as psU, \
                         tc.tile_pool(name="psD", bufs=2, space="PSUM") as psD:
                        for e in range(NEXP):
                            gu_sb = p2.tile(
                                [P, 16, TPC], BF16, name="gu_sb", tag="gu_sb", bufs=2
                            )
                            for m in range(16):
                                wg_m = p2.tile(
                                    [P, 8, P], BF16, name="wg_m", tag="wg_m", bufs=3
                                )
                                nc.sync.dma_start(
                                    out=wg_m[:], in_=wg[e, :, :, m * P : (m + 1) * P]
                                )
                                wu_m = p2.tile(
                                    [P, 8, P], BF16, name="wu_m", tag="wu_m", bufs=3
                                )
                                nc.sync.dma_start(
                                    out=wu_m[:], in_=wu[e, :, :, m * P : (m + 1) * P]
                                )
                                psg = psE.tile([P, TPC], FP32, name="psg", tag="psg")
                                psu = psU.tile([P, TPC], FP32, name="psu", tag="psu")
                                for k in range(8):
                                    nc.tensor.matmul(
                                        psg[:],
                                        wg_m[:, k, :],
                                        xf_bf[:, k, :],
                                        start=(k == 0),
                                        stop=(k == 7),
                                    )
                                for k in range(8):
                                    nc.tensor.matmul(
                                        psu[:],
                                        wu_m[:, k, :],
                                        xf_bf[:, k, :],
                                        start=(k == 0),
                                        stop=(k == 7),
                                    )
                                sg = p2.tile(
                                    [P, TPC], FP32, name="sg", tag="sg", bufs=2
                                )
                                nc.scalar.activation(sg[:], psg[:], ACTF.Silu, bias=zero_col[:])
                                nc.vector.tensor_tensor(
                                    gu_sb[:, m, :], sg[:], psu[:], op=ALU.mult
                                )
                            wd_e = p2.tile(
                                [P, 16, D], BF16, name="wd_e", tag="wd_e", bufs=1
                            )
                            nc.sync.dma_start(out=wd_e[:], in_=wd[e, :, :, :])
                            for mt in range(4):
                                for n2 in range(2):
                                    psd = psD.tile(
                                        [P, 512], FP32, name="psd", tag="psd"
                                    )
                                    for k2 in range(16):
                                        nc.tensor.matmul(
                                            psd[:],
                                            gu_sb[:, k2, mt * P : (mt + 1) * P],
                                            wd_e[:, k2, n2 * 512 : (n2 + 1) * 512],
                                            start=(k2 == 0),
                                            stop=(k2 == 15),
                                        )
                                    nc.vector.scalar_tensor_tensor(
                                        out=x1_sb[:, mt, n2 * 512 : (n2 + 1) * 512],
                                        in0=psd[:],
                                        scalar=wmat[:, mt, e : e + 1],
                                        in1=x1_sb[:, mt, n2 * 512 : (n2 + 1) * 512],
                                        op0=ALU.mult,
                                        op1=ALU.add,
                                    )
                # write output
                nc.sync.dma_start(
                    out=out_p.rearrange("(a p) d -> p a d", p=P), in_=x1_sb[:]
                )
    nc.compile()
    return nc


_NC_CACHE = None


def _get_nc():
    global _NC_CACHE
    if _NC_CACHE is None:
        _NC_CACHE = build()
    return _NC_CACHE


def _r(a, kp, *dims):
    """reshape [kp*P, ...] -> [P, kp, ...] weight layout for lhsT k-tiles."""
    a = np.ascontiguousarray(a)
    s = a.shape
    return np.ascontiguousarray(
        a.reshape(kp, P, *s[1:]).transpose(1, 0, *range(2, a.ndim + 1))
    )


def prep_in_maps(
    x, ssm_state, conv_state, ln1_s, ln1_b, ln2_s, ln2_b, in_proj_w, conv_w, conv_b,
    x_proj_w, dt_proj_w, dt_proj_b, A_log, Dp, out_proj_w, router_w, w_gate, w_up,
    w_down,
):
    f32 = np.float32
    bf16 = ml_dtypes.bfloat16
    x2 = np.asarray(x, f32).reshape(NTOK, D)
    w_in_z = _r(np.asarray(in_proj_w, f32)[DI:, :].T, 8)  # [128, 8, 2048]
    w_out = _r(np.asarray(out_proj_w, f32).T, 16)  # [128, 16, 1024]
    w_rt = _r(np.asarray(router_w, f32).T, 8)  # [128, 8, 8]
    wg_a = np.stack(
        [_r(np.asarray(w_gate[e], f32).T, 8).astype(bf16) for e in range(NEXP)]
    )
    wu_a = np.stack(
        [_r(np.asarray(w_up[e], f32).T, 8).astype(bf16) for e in range(NEXP)]
    )
    wd_a = np.stack(
        [_r(np.asarray(w_down[e], f32).T, 16).astype(bf16) for e in range(NEXP)]
    )
    ln = {
        "ln1_s": np.asarray(ln1_s, f32).reshape(1, D),
        "ln1_b": np.asarray(ln1_b, f32).reshape(1, D),
        "ln2_s": np.asarray(ln2_s, f32).reshape(1, D),
        "ln2_b": np.asarray(ln2_b, f32).reshape(1, D),
    }
    in_maps = []
    for c in range(NCORE):
        ch0, ch1 = c * CH, (c + 1) * CH
        tk0, tk1 = c * TPC, (c + 1) * TPC
        m = {
            "x_full": x2,
            "x_my": np.ascontiguousarray(x2[tk0:tk1]),
            "w_in_x": _r(np.asarray(in_proj_w, f32)[ch0:ch1, :].T, 8),
            "w_in_z": w_in_z,
            "w_xp": _r(np.asarray(x_proj_w, f32)[:, ch0:ch1].T, MT),
            "w_dt": np.ascontiguousarray(np.asarray(dt_proj_w, f32)[ch0:ch1, :].T),
            "w_out": w_out,
            "w_rt": w_rt,
            "wg": wg_a,
            "wu": wu_a,
            "wd": wd_a,
            "conv_w": _r(np.asarray(conv_w, f32)[ch0:ch1, 0, :], MT),
            "conv_b": _r(np.asarray(conv_b, f32)[ch0:ch1], MT),
            "dt_bias": _r(np.asarray(dt_proj_b, f32)[ch0:ch1], MT),
            "dp": _r(np.asarray(Dp, f32)[ch0:ch1], MT),
            "a_log": _r(np.asarray(A_log, f32)[ch0:ch1, :], MT),
            "ssm0": np.ascontiguousarray(
                np.asarray(ssm_state, f32)[:, ch0:ch1, :].reshape(Bb, MT, P, DS)
            ),
            "conv0": np.ascontiguousarray(
                np.asarray(conv_state, f32)[:, ch0:ch1, :].reshape(Bb, MT, P, 3)
            ),
            **ln,
        }
        in_maps.append(m)
    return in_maps


LAST_EXEC_NS = None


def kernel(**inputs):
    global LAST_EXEC_NS
    import os

    nc = _get_nc()
    in_maps = prep_in_maps(**inputs)
    kw = {}
    if os.environ.get("KERNEL_TRACE"):
        kw["trace"] = True
    res = run_bass_kernel_spmd(nc, in_maps, core_ids=list(range(NCORE)), **kw)
    LAST_EXEC_NS = getattr(res, "exec_time_ns", None)
    out = np.concatenate([res.results[c]["out"] for c in range(NCORE)], axis=0)
    return out.reshape(Bb, T, D).astype(np.float32)


if __name__ == "__main__":
    print("building...")
    nc = _get_nc()
    print("build ok")
